# revision 22
# baseline (speedup 1.0000x reference)
"""Multi-head attention (B=2, T=2048, D=OUT=1024, H=16) on 8 TRN2 NeuronCores.

Sharding: data-parallel over batch (2 groups) x tensor-parallel over heads
(4 groups of 4 heads). Core c handles batch c//4, heads (c%4)*4..(c%4)*4+4.
Each core computes Q^T/K^T/V for its head group, streams softmax(QK^T)V
in transposed layout (keys on partitions), and a partial output projection
through its W_o row block. The host sums the 4 partials per batch and adds
b_o.

Device-side layout notes:
- x is fed transposed ([D, T]) so Q^T/K^T come straight out of the PE.
- The softmax denominator rides along as a 65th ones-column in V', so one
  matmul produces both attn_unnorm^T and the per-query denominator.
- Softmax normalization multiplies attn^T (64 rows) by a partition-
  broadcast of 1/denom; every vector op keeps matching partition bases
  (the DVE cannot shift data across partitions).
- Matmul operands are bf16 (fp32 PSUM accumulation): full PE clock and
  fast weight load; fp32r measured 2x slower (the HAM clock gate never
  re-warms, and 4-byte LDWEIGHTS cannot use FWL).
- Every matmul contracts over K=128: K=64 matmuls do not register as PE
  activity for the HAM clock gate and run at half clock forever. The
  K-side operand of each attention matmul is a per-head tile zero-padded
  to 128 rows (even heads: data in rows 0:64; odd heads: rows 64:128,
  matching where the paired Q^T tile holds that head's rows).
- Emission is interleaved so the scalar engine's exp stream (the true
  bottleneck, ~146us) starts as soon as heads 0-1 are projected and the
  remaining projection/output matmuls fill the PE while exp runs.
"""

import numpy as np

import concourse.bass as bass
import concourse.mybir as mybir
import concourse.tile as tile
from concourse import bacc
from concourse.bass_utils import run_bass_kernel_spmd

B, T, D, OUT, H = 2, 2048, 1024, 1024, 16
DO = 256            # output columns per core (4 heads x 64)
DEPTH = 64
NH = 4              # heads per core
KT = D // 128       # 8 contraction tiles for the projections
TT = T // 128       # 16 key tiles
NB = T // 512       # 4 query/time blocks
F32 = mybir.dt.float32
BF16 = mybir.dt.bfloat16
MMDT = BF16
EXP = mybir.ActivationFunctionType.Exp
MULT = mybir.AluOpType.mult
ADD = mybir.AluOpType.add

_CACHE = {}


def build_attention(nc, dbg=None):
    """Emit the attention program. dbg, when given, is a dict of extra DRAM
    debug outputs to dump intermediates into."""
    xt = nc.declare_dram_parameter("xt", [D, T], MMDT, isOutput=False)
    wq = nc.declare_dram_parameter("wq", [D, DO], MMDT, isOutput=False)
    wk = nc.declare_dram_parameter("wk", [D, DO], MMDT, isOutput=False)
    wv = nc.declare_dram_parameter("wv", [D, DO], MMDT, isOutput=False)
    wo = nc.declare_dram_parameter("wo", [DEPTH, NH, OUT], MMDT, isOutput=False)
    bq2 = nc.declare_dram_parameter("bq2", [128, 2], F32, isOutput=False)
    bv = nc.declare_dram_parameter("bv", [DO], F32, isOutput=False)
    bcol = nc.declare_dram_parameter("bcol", [128, TT], F32, isOutput=False)
    outT = nc.declare_dram_parameter("outT", [OUT, T], F32, isOutput=True)

    with tile.TileContext(nc) as tc:
        with (
            tc.tile_pool(name="cw", bufs=1) as cw,
            tc.tile_pool(name="stage", bufs=2) as stage,
            tc.tile_pool(name="persist", bufs=1) as persist,
            tc.tile_pool(name="small", bufs=2) as small,
            tc.tile_pool(name="ptp", bufs=6) as ptp,
            tc.tile_pool(name="px", bufs=1) as px,
            tc.tile_pool(name="ps_s", bufs=3, space="PSUM") as ps_s,
            tc.tile_pool(name="ps_mm", bufs=1, space="PSUM") as ps_mm,
        ):
            # ---- constants ----
            bq_sb = cw.tile([128, 2], F32, tag="bq")
            nc.sync.dma_start(out=bq_sb[:], in_=bq2[:, :])
            bcol_sb = cw.tile([128, TT], F32, tag="bcol")
            nc.sync.dma_start(out=bcol_sb[:], in_=bcol[:, :])
            bv_sb = cw.tile([128, DO], F32, tag="bv")
            bv_ap = bv.ap()
            bv_bcast = bass.AP(tensor=bv_ap.tensor, offset=bv_ap.offset, ap=[[0, 128], [1, DO]])
            nc.sync.dma_start(out=bv_sb[:], in_=bv_bcast)
            ones_f = cw.tile([128, NH], F32, tag="ones")
            nc.vector.memset(ones_f[:], 1.0)

            # ---- inputs (bf16 straight from the host) ----
            def load_bf16(pool, dram_ap, shape, tag):
                r = pool.tile(shape, MMDT, tag=tag, name=f"r_{tag}")
                nc.sync.dma_start(out=r[:], in_=dram_ap)
                return r

            wq_r = load_bf16(px, wq.rearrange("(kt p) m -> p kt m", p=128), [128, KT, DO], "wq")
            wk_r = load_bf16(px, wk.rearrange("(kt p) m -> p kt m", p=128), [128, KT, DO], "wk")
            wv_r = load_bf16(px, wv.rearrange("(kt p) m -> p kt m", p=128), [128, KT, DO], "wv")
            xr = []
            for kt in range(KT):
                xr.append(load_bf16(px, xt[kt * 128:(kt + 1) * 128, :], [128, T], f"xr{kt}"))

            # ---- persistent activation tiles ----
            qt2 = [persist.tile([128, T], MMDT, tag=f"qt{mi}", name=f"qt{mi}") for mi in range(2)]
            kth = [persist.tile([128, T], MMDT, tag=f"kh{h}", name=f"kh{h}") for h in range(NH)]
            vp = persist.tile([128, TT, NH * 65], MMDT, tag="vp")
            at4 = [persist.tile([128, T], MMDT, tag=f"at{h}", name=f"at{h}") for h in range(NH)]
            for h in range(NH):
                lo, hi = ((64, 128) if h % 2 == 0 else (0, 64))
                nc.vector.memset(kth[h][lo:hi, :], 0.0)
                nc.vector.memset(at4[h][64:128, :], 0.0)

            # ---- emission helpers (generators advanced by the interleaver) ----
            def qk_group(w_r, mi, nb, dst):
                """One [128,512] projection psum group: 8 matmuls + biased copy."""
                ps = ps_s.tile([128, 1024], F32, tag="s", name=f"ps_p{mi}_{nb}")
                for kt in range(KT):
                    nc.tensor.matmul(
                        ps[:, :512],
                        w_r[:, kt, mi * 128:(mi + 1) * 128],
                        xr[kt][:, nb * 512:(nb + 1) * 512],
                        start=(kt == 0),
                        stop=(kt == KT - 1),
                    )
                nc.vector.tensor_scalar_add(
                    dst[mi][:, nb * 512:(nb + 1) * 512], ps[:, :512], bq_sb[:, mi:mi + 1]
                )

            kt2 = [None, None]  # paired K^T staging (rows then split into kth)

            def build_kth(mi):
                # even head of the pair: rows 0:64 stay, odd head: rows 64:128
                h0, h1 = 2 * mi, 2 * mi + 1
                nc.vector.tensor_copy(out=kth[h0][0:64, :], in_=kt2[mi][0:64, :])
                nc.vector.tensor_copy(out=kth[h1][64:128, :], in_=kt2[mi][64:128, :])

            def v_group(tt):
                ps = ps_s.tile([128, 1024], F32, tag="s", name=f"ps_v{tt}")
                for kt in range(KT):
                    nc.tensor.matmul(
                        ps[:, :DO],
                        xr[kt][:, tt * 128:(tt + 1) * 128],
                        wv_r[:, kt, :],
                        start=(kt == 0),
                        stop=(kt == KT - 1),
                    )
                for h in range(NH):
                    nc.vector.tensor_tensor(
                        vp[:, tt, h * 65:h * 65 + 64],
                        ps[:, h * 64:(h + 1) * 64],
                        bv_sb[:, h * 64:(h + 1) * 64],
                        ADD,
                    )
                ones_ap = vp[:, tt, :].rearrange("p (h c) -> p h c", c=65)[:, :, 64:65]
                nc.gpsimd.tensor_copy(out=ones_ap, in_=ones_f[:, :, None])

            wo_r = []

            def load_wo():
                for hh in range(2):
                    r = ptp.tile([128, 2, OUT], MMDT, tag=f"wo{hh}", name=f"r_wo{hh}")
                    nc.sync.dma_start(out=r[0:64, :, :], in_=wo[:, 2 * hh:2 * hh + 2, :])
                    nc.vector.memset(r[64:128, :, :], 0.0)
                    wo_r.append(r)

            def attn_unit(qbp, h, kt, attn_ps, prev_pv):
                """One attention inner step: 2 S-matmuls, exp, PV of prev."""
                s_ps = ps_s.tile([128, 1024], F32, tag="s", name=f"s_{qbp}_{h}_{kt}")
                for half in range(2):
                    nc.tensor.matmul(
                        s_ps[:, half * 512:(half + 1) * 512],
                        kth[h][:, kt * 128:(kt + 1) * 128],
                        qt2[h // 2][:, qbp * 1024 + half * 512:qbp * 1024 + (half + 1) * 512],
                        start=True,
                        stop=True,
                    )
                pt = ptp.tile([128, 1024], MMDT, tag="pt")
                nc.scalar.activation(
                    pt[:], s_ps[:], EXP, bias=bcol_sb[:, kt:kt + 1], scale=0.125
                )
                if prev_pv is not None:
                    emit_pv(h, attn_ps, *prev_pv)
                return pt

            def emit_pv(h, attn_ps, kt, pt):
                for half in range(2):
                    nc.tensor.matmul(
                        attn_ps[:65, half * 512:(half + 1) * 512],
                        vp[:, kt, h * 65:(h + 1) * 65],
                        pt[:, half * 512:(half + 1) * 512],
                        start=(kt == 0),
                        stop=(kt == TT - 1),
                    )

            def normalize(qbp, h, attn_ps):
                den = small.tile([65, 1024], F32, tag="den", name=f"den{qbp}_{h}")
                nc.vector.tensor_copy(out=den[64:65, :], in_=attn_ps[64:65, :])
                d0 = small.tile([1, 1024], F32, tag="d0", name=f"d0{qbp}_{h}")
                nc.sync.dma_start(out=d0[:], in_=den[64:65, :])
                rec = small.tile([1, 1024], F32, tag="rec", name=f"rec{qbp}_{h}")
                nc.vector.reciprocal_approx_fast(rec[:], d0[:])
                rb = small.tile([64, 1024], F32, tag="rb", name=f"rb{qbp}_{h}")
                nc.gpsimd.partition_broadcast(rb[:], rec[:])
                nc.vector.tensor_tensor(
                    at4[h][0:64, qbp * 1024:(qbp + 1) * 1024],
                    attn_ps[0:64, :],
                    rb[:],
                    MULT,
                )

            def attn_head(qbp, h, filler=None):
                """Full head block; filler() is called once per kt iteration
                to interleave independent PE work into the exp-bound loop."""
                attn_ps = ps_mm.tile([128, 1024], F32, tag="attn", name=f"attn_{qbp}_{h}")
                prev = None
                for kt in range(TT):
                    pt = attn_unit(qbp, h, kt, attn_ps, prev)
                    prev = (kt, pt)
                    if filler is not None:
                        filler(kt)
                emit_pv(h, attn_ps, *prev)
                normalize(qbp, h, attn_ps)

            def c_group(nt, tb):
                ps = ps_s.tile([128, 1024], F32, tag="s", name=f"ps_c{nt}_{tb}")
                for h in range(NH):
                    nc.tensor.matmul(
                        ps[:, :512],
                        wo_r[h // 2][:, h % 2, nt * 128:(nt + 1) * 128],
                        at4[h][:, tb * 512:(tb + 1) * 512],
                        start=(h == 0),
                        stop=(h == NH - 1),
                    )
                o_sb = stage.tile([128, 512], F32, tag="stage", name="o_sb")
                nc.vector.tensor_copy(out=o_sb[:], in_=ps[:, :512])
                nc.sync.dma_start(
                    out=outT[nt * 128:(nt + 1) * 128, tb * 512:(tb + 1) * 512],
                    in_=o_sb[:],
                )

            # ---- emission schedule ----
            # heads 0-1 inputs first so the exp stream starts early; V'
            # tiles and the mi=1 projections are emitted just-in-time as
            # fillers inside the first two heads' exp-bound loops
            kt2[0] = persist.tile([128, T], MMDT, tag="kt2a", name="kt2a")
            kt2[1] = persist.tile([128, T], MMDT, tag="kt2b", name="kt2b")
            for nb in range(NB):
                qk_group(wq_r, 0, nb, qt2)
            for nb in range(NB):
                qk_group(wk_r, 0, nb, kt2)
            build_kth(0)
            v_group(0)

            fill_work = []
            for tt in range(1, TT):
                fill_work.append(lambda tt=tt: v_group(tt))
            for nb in range(NB):
                fill_work.append(lambda nb=nb: qk_group(wq_r, 1, nb, qt2))
            for nb in range(NB):
                fill_work.append(lambda nb=nb: qk_group(wk_r, 1, nb, kt2))
            fill_work.append(lambda: build_kth(1))
            fill_work.append(load_wo)

            def filler(kt):
                if fill_work:
                    fill_work.pop(0)()

            attn_head(0, 0, filler)
            attn_head(0, 1, filler)
            while fill_work:
                fill_work.pop(0)()
            attn_head(0, 2)
            attn_head(0, 3)

            # qbp=1 heads, with the first half of the output projection
            # interleaved (at4[:, 0:1024] is complete after qbp=0)
            c_work = [(nt, tb) for tb in range(2) for nt in range(OUT // 128)]

            def filler_c(kt):
                if c_work and kt % 2 == 0:
                    nt, tb = c_work.pop(0)
                    c_group(nt, tb)

            attn_head(1, 0, filler_c)
            attn_head(1, 1, filler_c)
            attn_head(1, 2, filler_c)
            attn_head(1, 3, filler_c)
            while c_work:
                nt, tb = c_work.pop(0)
                c_group(nt, tb)
            for tb in range(2, NB):
                for nt in range(OUT // 128):
                    c_group(nt, tb)

            if dbg:
                def dump32(dst, src_ap, shape, nm):
                    t = stage.tile(shape, F32, tag="dump", name=f"dump_{nm}")
                    nc.vector.tensor_copy(out=t[:], in_=src_ap)
                    nc.sync.dma_start(out=dst, in_=t[:])
                for h in range(NH):
                    dump32(dbg["d_qt"][h], qt2[h // 2][(h % 2) * 64:(h % 2) * 64 + 64, :], [64, T], f"qt{h}")
                    kt_rows = kth[h][0:64, :] if h % 2 == 0 else kth[h][64:128, :]
                    dump32(dbg["d_kt"][h], kt_rows, [64, T], f"kt{h}")
                    dump32(dbg["d_at"][h], at4[h][0:64, :], [64, T], f"at{h}")
                dump32(dbg["d_vp"][:, :, :], vp[:], [128, TT, NH * 65], "vp")


def _build():
    nc = bacc.Bacc(trn_type="TRN2")
    build_attention(nc)
    nc.compile()
    return nc


def _get_nc():
    if "nc" not in _CACHE:
        _CACHE["nc"] = _build()
    return _CACHE["nc"]


def make_in_maps(x, W_q, b_q, W_k, W_v, b_v, W_o, bias):
    import ml_dtypes
    bf16 = ml_dtypes.bfloat16
    in_maps = []
    xtb = [np.ascontiguousarray(x[b].T.astype(bf16)) for b in range(B)]
    wqb = W_q.astype(bf16)
    wkb = W_k.astype(bf16)
    wvb = W_v.astype(bf16)
    wob = W_o.astype(bf16)
    for c in range(8):
        b, hg = divmod(c, 4)
        sl = slice(hg * DO, (hg + 1) * DO)
        in_maps.append({
            "xt": xtb[b],
            "wq": np.ascontiguousarray(wqb[:, sl]),
            "wk": np.ascontiguousarray(wkb[:, sl]),
            "wv": np.ascontiguousarray(wvb[:, sl]),
            "wo": np.ascontiguousarray(wob[sl, :].reshape(NH, DEPTH, OUT).transpose(1, 0, 2)),
            "bq2": np.ascontiguousarray(b_q[sl].reshape(2, 128).T),
            "bv": np.ascontiguousarray(b_v[sl]),
            "bcol": np.ascontiguousarray(bias.reshape(TT, 128).T),
        })
    return in_maps


def kernel(x, W_q, b_q, W_k, b_k, W_v, b_v, W_o, b_o, bias, **_ignored):
    x = np.asarray(x, dtype=np.float32)
    W_q = np.asarray(W_q, dtype=np.float32)
    W_k = np.asarray(W_k, dtype=np.float32)
    W_v = np.asarray(W_v, dtype=np.float32)
    W_o = np.asarray(W_o, dtype=np.float32)
    b_q = np.asarray(b_q, dtype=np.float32)
    b_v = np.asarray(b_v, dtype=np.float32)
    b_o = np.asarray(b_o, dtype=np.float32)
    bias = np.asarray(bias, dtype=np.float32)

    nc = _get_nc()
    in_maps = make_in_maps(x, W_q, b_q, W_k, W_v, b_v, W_o, bias)
    _CACHE["in_maps"] = in_maps
    res = run_bass_kernel_spmd(nc, in_maps, list(range(8)))
    out = np.zeros((B, T, OUT), dtype=np.float32)
    for c in range(8):
        out[c // 4] += res.results[c]["outT"].T
    out += b_o
    return out


# revision 23
# speedup vs baseline: 1.0365x; 1.0365x over previous
"""Multi-head attention (B=2, T=2048, D=OUT=1024, H=16) on 8 TRN2 NeuronCores.

Sharding: data-parallel over batch (2 groups) x tensor-parallel over heads
(4 groups of 4 heads). Core c handles batch c//4, heads (c%4)*4..(c%4)*4+4.
Each core computes Q^T/K^T/V for its head group, streams softmax(QK^T)V
in transposed layout (keys on partitions), and a partial output projection
through its W_o row block. The host sums the 4 partials per batch and adds
b_o.

Device-side layout notes:
- x is fed transposed ([D, T]) so Q^T/K^T come straight out of the PE.
- The softmax denominator rides along as a 65th ones-column in V', so one
  matmul produces both attn_unnorm^T and the per-query denominator.
- Softmax normalization multiplies attn^T (64 rows) by a partition-
  broadcast of 1/denom; every vector op keeps matching partition bases
  (the DVE cannot shift data across partitions).
- Matmul operands are bf16 (fp32 PSUM accumulation): full PE clock and
  fast weight load; fp32r measured 2x slower (the HAM clock gate never
  re-warms, and 4-byte LDWEIGHTS cannot use FWL).
- Every matmul contracts over K=128: K=64 matmuls do not register as PE
  activity for the HAM clock gate and run at half clock forever. The
  K-side operand of each attention matmul is a per-head tile zero-padded
  to 128 rows (even heads: data in rows 0:64; odd heads: rows 64:128,
  matching where the paired Q^T tile holds that head's rows).
- Emission is interleaved so the scalar engine's exp stream (the true
  bottleneck, ~146us) starts as soon as heads 0-1 are projected and the
  remaining projection/output matmuls fill the PE while exp runs.
"""

import numpy as np

import concourse.bass as bass
import concourse.mybir as mybir
import concourse.tile as tile
from concourse import bacc
from concourse.bass_utils import run_bass_kernel_spmd

B, T, D, OUT, H = 2, 2048, 1024, 1024, 16
DO = 256            # output columns per core (4 heads x 64)
DEPTH = 64
NH = 4              # heads per core
KT = D // 128       # 8 contraction tiles for the projections
TT = T // 128       # 16 key tiles
NB = T // 512       # 4 query/time blocks
F32 = mybir.dt.float32
BF16 = mybir.dt.bfloat16
MMDT = BF16
EXP = mybir.ActivationFunctionType.Exp
MULT = mybir.AluOpType.mult
ADD = mybir.AluOpType.add

_CACHE = {}


def build_attention(nc, dbg=None):
    """Emit the attention program. dbg, when given, is a dict of extra DRAM
    debug outputs to dump intermediates into."""
    xt = nc.declare_dram_parameter("xt", [D, T], MMDT, isOutput=False)
    wq = nc.declare_dram_parameter("wq", [D, DO], MMDT, isOutput=False)
    wk = nc.declare_dram_parameter("wk", [D, DO], MMDT, isOutput=False)
    wv = nc.declare_dram_parameter("wv", [D, DO], MMDT, isOutput=False)
    wo = nc.declare_dram_parameter("wo", [DEPTH, NH, OUT], MMDT, isOutput=False)
    bq2 = nc.declare_dram_parameter("bq2", [128, 2], F32, isOutput=False)
    bv = nc.declare_dram_parameter("bv", [DO], F32, isOutput=False)
    bcol = nc.declare_dram_parameter("bcol", [128, TT], F32, isOutput=False)
    outT = nc.declare_dram_parameter("outT", [OUT, T], F32, isOutput=True)

    with tile.TileContext(nc) as tc:
        with (
            tc.tile_pool(name="cw", bufs=1) as cw,
            tc.tile_pool(name="stage", bufs=2) as stage,
            tc.tile_pool(name="persist", bufs=1) as persist,
            tc.tile_pool(name="small", bufs=2) as small,
            tc.tile_pool(name="ptp", bufs=6) as ptp,
            tc.tile_pool(name="px", bufs=1) as px,
            tc.tile_pool(name="ps_s", bufs=2, space="PSUM") as ps_s,
            tc.tile_pool(name="ps_mm", bufs=1, space="PSUM") as ps_mm,
        ):
            # ---- constants ----
            bq_sb = cw.tile([128, 2], F32, tag="bq")
            nc.sync.dma_start(out=bq_sb[:], in_=bq2[:, :])
            bcol_sb = cw.tile([128, TT], F32, tag="bcol")
            nc.sync.dma_start(out=bcol_sb[:], in_=bcol[:, :])
            bv_sb = cw.tile([128, DO], F32, tag="bv")
            bv_ap = bv.ap()
            bv_bcast = bass.AP(tensor=bv_ap.tensor, offset=bv_ap.offset, ap=[[0, 128], [1, DO]])
            nc.sync.dma_start(out=bv_sb[:], in_=bv_bcast)
            ones_f = cw.tile([128, NH], F32, tag="ones")
            nc.vector.memset(ones_f[:], 1.0)

            # ---- inputs (bf16 straight from the host) ----
            def load_bf16(pool, dram_ap, shape, tag):
                r = pool.tile(shape, MMDT, tag=tag, name=f"r_{tag}")
                nc.sync.dma_start(out=r[:], in_=dram_ap)
                return r

            wq_r = load_bf16(px, wq.rearrange("(kt p) m -> p kt m", p=128), [128, KT, DO], "wq")
            wk_r = load_bf16(px, wk.rearrange("(kt p) m -> p kt m", p=128), [128, KT, DO], "wk")
            wv_r = load_bf16(px, wv.rearrange("(kt p) m -> p kt m", p=128), [128, KT, DO], "wv")
            xr = []
            for kt in range(KT):
                xr.append(load_bf16(px, xt[kt * 128:(kt + 1) * 128, :], [128, T], f"xr{kt}"))

            # ---- persistent activation tiles ----
            qt2 = [persist.tile([128, T], MMDT, tag=f"qt{mi}", name=f"qt{mi}") for mi in range(2)]
            kth = [persist.tile([128, T], MMDT, tag=f"kh{h}", name=f"kh{h}") for h in range(NH)]
            vp = persist.tile([128, TT, NH * 65], MMDT, tag="vp")
            at4 = [persist.tile([128, T], MMDT, tag=f"at{h}", name=f"at{h}") for h in range(NH)]
            for h in range(NH):
                lo, hi = ((64, 128) if h % 2 == 0 else (0, 64))
                nc.vector.memset(kth[h][lo:hi, :], 0.0)
                nc.vector.memset(at4[h][64:128, :], 0.0)

            # ---- emission helpers (generators advanced by the interleaver) ----
            def qk_group(w_r, mi, nb, dst):
                """One [128,512] projection psum group: 8 matmuls + biased copy."""
                ps = ps_mm.tile([128, 1024], F32, tag="fill", name=f"ps_p{mi}_{nb}")
                for kt in range(KT):
                    nc.tensor.matmul(
                        ps[:, :512],
                        w_r[:, kt, mi * 128:(mi + 1) * 128],
                        xr[kt][:, nb * 512:(nb + 1) * 512],
                        start=(kt == 0),
                        stop=(kt == KT - 1),
                    )
                nc.vector.tensor_scalar_add(
                    dst[mi][:, nb * 512:(nb + 1) * 512], ps[:, :512], bq_sb[:, mi:mi + 1]
                )

            kt2 = [None, None]  # paired K^T staging (rows then split into kth)

            def build_kth(mi):
                # even head of the pair: rows 0:64 stay, odd head: rows 64:128
                h0, h1 = 2 * mi, 2 * mi + 1
                nc.vector.tensor_copy(out=kth[h0][0:64, :], in_=kt2[mi][0:64, :])
                nc.vector.tensor_copy(out=kth[h1][64:128, :], in_=kt2[mi][64:128, :])

            def v_group(tt):
                ps = ps_mm.tile([128, 1024], F32, tag="fill", name=f"ps_v{tt}")
                for kt in range(KT):
                    nc.tensor.matmul(
                        ps[:, :DO],
                        xr[kt][:, tt * 128:(tt + 1) * 128],
                        wv_r[:, kt, :],
                        start=(kt == 0),
                        stop=(kt == KT - 1),
                    )
                for h in range(NH):
                    nc.vector.tensor_tensor(
                        vp[:, tt, h * 65:h * 65 + 64],
                        ps[:, h * 64:(h + 1) * 64],
                        bv_sb[:, h * 64:(h + 1) * 64],
                        ADD,
                    )
                ones_ap = vp[:, tt, :].rearrange("p (h c) -> p h c", c=65)[:, :, 64:65]
                nc.gpsimd.tensor_copy(out=ones_ap, in_=ones_f[:, :, None])

            wo_r = []

            def load_wo():
                for hh in range(2):
                    r = ptp.tile([128, 2, OUT], MMDT, tag=f"wo{hh}", name=f"r_wo{hh}")
                    nc.sync.dma_start(out=r[0:64, :, :], in_=wo[:, 2 * hh:2 * hh + 2, :])
                    nc.vector.memset(r[64:128, :, :], 0.0)
                    wo_r.append(r)

            def attn_unit(qbp, h, kt, attn_ps, prev_pv):
                """One attention inner step: 2 S-matmuls, exp, PV of prev."""
                s_ps = ps_s.tile([128, 1024], F32, tag="s", name=f"s_{qbp}_{h}_{kt}")
                for half in range(2):
                    nc.tensor.matmul(
                        s_ps[:, half * 512:(half + 1) * 512],
                        kth[h][:, kt * 128:(kt + 1) * 128],
                        qt2[h // 2][:, qbp * 1024 + half * 512:qbp * 1024 + (half + 1) * 512],
                        start=True,
                        stop=True,
                    )
                pt = ptp.tile([128, 1024], MMDT, tag="pt")
                nc.scalar.activation(
                    pt[:], s_ps[:], EXP, bias=bcol_sb[:, kt:kt + 1], scale=0.125
                )
                if prev_pv is not None:
                    emit_pv(h, attn_ps, *prev_pv)
                return pt

            def emit_pv(h, attn_ps, kt, pt):
                for half in range(2):
                    nc.tensor.matmul(
                        attn_ps[:65, half * 512:(half + 1) * 512],
                        vp[:, kt, h * 65:(h + 1) * 65],
                        pt[:, half * 512:(half + 1) * 512],
                        start=(kt == 0),
                        stop=(kt == TT - 1),
                    )

            def normalize(qbp, h, attn_ps):
                den = small.tile([65, 1024], F32, tag="den", name=f"den{qbp}_{h}")
                nc.vector.tensor_copy(out=den[64:65, :], in_=attn_ps[64:65, :])
                d0 = small.tile([1, 1024], F32, tag="d0", name=f"d0{qbp}_{h}")
                nc.sync.dma_start(out=d0[:], in_=den[64:65, :])
                rec = small.tile([1, 1024], F32, tag="rec", name=f"rec{qbp}_{h}")
                nc.vector.reciprocal_approx_fast(rec[:], d0[:])
                rb = small.tile([64, 1024], F32, tag="rb", name=f"rb{qbp}_{h}")
                nc.gpsimd.partition_broadcast(rb[:], rec[:])
                nc.vector.tensor_tensor(
                    at4[h][0:64, qbp * 1024:(qbp + 1) * 1024],
                    attn_ps[0:64, :],
                    rb[:],
                    MULT,
                )

            def attn_head(qbp, h, filler=None):
                """Full head block; filler() is called once per kt iteration
                to interleave independent PE work into the exp-bound loop."""
                attn_ps = ps_mm.tile([128, 1024], F32, tag="attn", name=f"attn_{qbp}_{h}")
                prev = None
                for kt in range(TT):
                    pt = attn_unit(qbp, h, kt, attn_ps, prev)
                    prev = (kt, pt)
                    if filler is not None:
                        filler(kt)
                emit_pv(h, attn_ps, *prev)
                normalize(qbp, h, attn_ps)

            def c_group(nt, tb):
                ps = ps_mm.tile([128, 1024], F32, tag="fill", name=f"ps_c{nt}_{tb}")
                for h in range(NH):
                    nc.tensor.matmul(
                        ps[:, :512],
                        wo_r[h // 2][:, h % 2, nt * 128:(nt + 1) * 128],
                        at4[h][:, tb * 512:(tb + 1) * 512],
                        start=(h == 0),
                        stop=(h == NH - 1),
                    )
                o_sb = stage.tile([128, 512], F32, tag="stage", name="o_sb")
                nc.vector.tensor_copy(out=o_sb[:], in_=ps[:, :512])
                nc.sync.dma_start(
                    out=outT[nt * 128:(nt + 1) * 128, tb * 512:(tb + 1) * 512],
                    in_=o_sb[:],
                )

            # ---- emission schedule ----
            # heads 0-1 inputs first so the exp stream starts early; V'
            # tiles and the mi=1 projections are emitted just-in-time as
            # fillers inside the first two heads' exp-bound loops
            kt2[0] = persist.tile([128, T], MMDT, tag="kt2a", name="kt2a")
            kt2[1] = persist.tile([128, T], MMDT, tag="kt2b", name="kt2b")
            for nb in range(NB):
                qk_group(wq_r, 0, nb, qt2)
            for nb in range(NB):
                qk_group(wk_r, 0, nb, kt2)
            build_kth(0)
            v_group(0)

            fill_work = []
            for tt in range(1, TT):
                fill_work.append(lambda tt=tt: v_group(tt))
            for nb in range(NB):
                fill_work.append(lambda nb=nb: qk_group(wq_r, 1, nb, qt2))
            for nb in range(NB):
                fill_work.append(lambda nb=nb: qk_group(wk_r, 1, nb, kt2))
            fill_work.append(lambda: build_kth(1))
            fill_work.append(load_wo)

            def filler(kt):
                if fill_work:
                    fill_work.pop(0)()

            attn_head(0, 0, filler)
            attn_head(0, 1, filler)
            while fill_work:
                fill_work.pop(0)()
            attn_head(0, 2)
            attn_head(0, 3)

            # qbp=1 heads, with the first half of the output projection
            # interleaved (at4[:, 0:1024] is complete after qbp=0)
            c_work = [(nt, tb) for tb in range(2) for nt in range(OUT // 128)]

            def filler_c(kt):
                if c_work and kt % 2 == 0:
                    nt, tb = c_work.pop(0)
                    c_group(nt, tb)

            attn_head(1, 0, filler_c)
            attn_head(1, 1, filler_c)
            attn_head(1, 2, filler_c)
            attn_head(1, 3, filler_c)
            while c_work:
                nt, tb = c_work.pop(0)
                c_group(nt, tb)
            for tb in range(2, NB):
                for nt in range(OUT // 128):
                    c_group(nt, tb)

            if dbg:
                def dump32(dst, src_ap, shape, nm):
                    t = stage.tile(shape, F32, tag="dump", name=f"dump_{nm}")
                    nc.vector.tensor_copy(out=t[:], in_=src_ap)
                    nc.sync.dma_start(out=dst, in_=t[:])
                for h in range(NH):
                    dump32(dbg["d_qt"][h], qt2[h // 2][(h % 2) * 64:(h % 2) * 64 + 64, :], [64, T], f"qt{h}")
                    kt_rows = kth[h][0:64, :] if h % 2 == 0 else kth[h][64:128, :]
                    dump32(dbg["d_kt"][h], kt_rows, [64, T], f"kt{h}")
                    dump32(dbg["d_at"][h], at4[h][0:64, :], [64, T], f"at{h}")
                dump32(dbg["d_vp"][:, :, :], vp[:], [128, TT, NH * 65], "vp")


def _build():
    nc = bacc.Bacc(trn_type="TRN2")
    build_attention(nc)
    nc.compile()
    return nc


def _get_nc():
    if "nc" not in _CACHE:
        _CACHE["nc"] = _build()
    return _CACHE["nc"]


def make_in_maps(x, W_q, b_q, W_k, W_v, b_v, W_o, bias):
    import ml_dtypes
    bf16 = ml_dtypes.bfloat16
    in_maps = []
    xtb = [np.ascontiguousarray(x[b].T.astype(bf16)) for b in range(B)]
    wqb = W_q.astype(bf16)
    wkb = W_k.astype(bf16)
    wvb = W_v.astype(bf16)
    wob = W_o.astype(bf16)
    for c in range(8):
        b, hg = divmod(c, 4)
        sl = slice(hg * DO, (hg + 1) * DO)
        in_maps.append({
            "xt": xtb[b],
            "wq": np.ascontiguousarray(wqb[:, sl]),
            "wk": np.ascontiguousarray(wkb[:, sl]),
            "wv": np.ascontiguousarray(wvb[:, sl]),
            "wo": np.ascontiguousarray(wob[sl, :].reshape(NH, DEPTH, OUT).transpose(1, 0, 2)),
            "bq2": np.ascontiguousarray(b_q[sl].reshape(2, 128).T),
            "bv": np.ascontiguousarray(b_v[sl]),
            "bcol": np.ascontiguousarray(bias.reshape(TT, 128).T),
        })
    return in_maps


def kernel(x, W_q, b_q, W_k, b_k, W_v, b_v, W_o, b_o, bias, **_ignored):
    x = np.asarray(x, dtype=np.float32)
    W_q = np.asarray(W_q, dtype=np.float32)
    W_k = np.asarray(W_k, dtype=np.float32)
    W_v = np.asarray(W_v, dtype=np.float32)
    W_o = np.asarray(W_o, dtype=np.float32)
    b_q = np.asarray(b_q, dtype=np.float32)
    b_v = np.asarray(b_v, dtype=np.float32)
    b_o = np.asarray(b_o, dtype=np.float32)
    bias = np.asarray(bias, dtype=np.float32)

    nc = _get_nc()
    in_maps = make_in_maps(x, W_q, b_q, W_k, W_v, b_v, W_o, bias)
    _CACHE["in_maps"] = in_maps
    res = run_bass_kernel_spmd(nc, in_maps, list(range(8)))
    out = np.zeros((B, T, OUT), dtype=np.float32)
    for c in range(8):
        out[c // 4] += res.results[c]["outT"].T
    out += b_o
    return out


# revision 44
# speedup vs baseline: 1.2509x; 1.2069x over previous
"""Multi-head attention (B=2, T=2048, D=OUT=1024, H=16) on 8 TRN2 NeuronCores.

Sharding: data-parallel over batch (2 groups) x tensor-parallel over heads
(4 groups of 4 heads). Core c handles batch c//4, heads (c%4)*4..(c%4)*4+4.
Each core computes Q^T/K^T/V for its head group, streams softmax(QK^T)V
in transposed layout (keys on partitions), and a partial output projection
through its W_o row block. The host sums the 4 partials per batch and adds
b_o.

Device-side layout notes:
- x is fed transposed ([D, T]) so Q^T/K^T come straight out of the PE.
- The softmax denominator rides along as a 65th ones-column in V', so one
  matmul produces both attn_unnorm^T and the per-query denominator.
- Softmax normalization multiplies attn^T (64 rows) by a partition-
  broadcast of 1/denom; every vector op keeps matching partition bases
  (the DVE cannot shift data across partitions).
- Matmul operands are bf16 (fp32 PSUM accumulation): full PE clock and
  fast weight load; fp32r measured 2x slower (the HAM clock gate never
  re-warms, and 4-byte LDWEIGHTS cannot use FWL).
- Every matmul contracts over K=128: K=64 matmuls do not register as PE
  activity for the HAM clock gate and run at half clock forever. The
  K-side operand of each attention matmul is a per-head tile zero-padded
  to 128 rows (even heads: data in rows 0:64; odd heads: rows 64:128,
  matching where the paired Q^T tile holds that head's rows).
- Emission is interleaved so the scalar engine's exp stream (the true
  bottleneck, ~146us) starts as soon as heads 0-1 are projected and the
  remaining projection/output matmuls fill the PE while exp runs.
"""

import numpy as np

import concourse.bass as bass
import concourse.mybir as mybir
import concourse.tile as tile
from concourse import bacc
from concourse.bass_utils import run_bass_kernel_spmd

B, T, D, OUT, H = 2, 2048, 1024, 1024, 16
DO = 256            # output columns per core (4 heads x 64)
DEPTH = 64
NH = 4              # heads per core
KT = D // 128       # 8 contraction tiles for the projections
TT = T // 128       # 16 key tiles
NB = T // 512       # 4 query/time blocks
F32 = mybir.dt.float32
BF16 = mybir.dt.bfloat16
MMDT = BF16
EXP = mybir.ActivationFunctionType.Exp
MULT = mybir.AluOpType.mult
ADD = mybir.AluOpType.add

_CACHE = {}


def build_attention(nc, dbg=None):
    """Emit the attention program. dbg, when given, is a dict of extra DRAM
    debug outputs to dump intermediates into."""
    xt = nc.declare_dram_parameter("xt", [D, T], MMDT, isOutput=False)
    wq = nc.declare_dram_parameter("wq", [D, DO], MMDT, isOutput=False)
    wk = nc.declare_dram_parameter("wk", [D, DO], MMDT, isOutput=False)
    wv = nc.declare_dram_parameter("wv", [D, DO], MMDT, isOutput=False)
    wo = nc.declare_dram_parameter("wo", [DEPTH, NH, OUT], MMDT, isOutput=False)
    bq2 = nc.declare_dram_parameter("bq2", [128, 2], F32, isOutput=False)
    bv = nc.declare_dram_parameter("bv", [DO], F32, isOutput=False)
    bcol = nc.declare_dram_parameter("bcol", [128, TT], F32, isOutput=False)
    outT = nc.declare_dram_parameter("outT", [OUT, T], F32, isOutput=True)

    with tile.TileContext(nc) as tc:
        with (
            tc.tile_pool(name="cw", bufs=1) as cw,
            tc.tile_pool(name="stage", bufs=6) as stage,
            tc.tile_pool(name="persist", bufs=1) as persist,
            tc.tile_pool(name="small", bufs=2) as small,
            tc.tile_pool(name="ptp", bufs=(4 if dbg else 6)) as ptp,
            tc.tile_pool(name="px", bufs=1) as px,
            tc.tile_pool(name="ps_s", bufs=2, space="PSUM") as ps_s,
            tc.tile_pool(name="ps_mm", bufs=2, space="PSUM") as ps_mm,
        ):
            # ---- constants ----
            bq_sb = cw.tile([128, 2], F32, tag="bq")
            nc.sync.dma_start(out=bq_sb[:], in_=bq2[:, :])
            bcol_sb = cw.tile([128, TT], F32, tag="bcol")
            nc.sync.dma_start(out=bcol_sb[:], in_=bcol[:, :])
            bv_sb = cw.tile([128, DO], F32, tag="bv")
            bv_ap = bv.ap()
            bv_bcast = bass.AP(tensor=bv_ap.tensor, offset=bv_ap.offset, ap=[[0, 128], [1, DO]])
            nc.sync.dma_start(out=bv_sb[:], in_=bv_bcast)
            ones_f = cw.tile([128, NH], F32, tag="ones")
            nc.vector.memset(ones_f[:], 1.0)

            # ---- inputs (bf16 straight from the host) ----
            def load_bf16(pool, dram_ap, shape, tag):
                r = pool.tile(shape, MMDT, tag=tag, name=f"r_{tag}")
                nc.sync.dma_start(out=r[:], in_=dram_ap)
                return r

            wq_r = load_bf16(px, wq.rearrange("(kt p) m -> p kt m", p=128), [128, KT, DO], "wq")
            wk_r = load_bf16(px, wk.rearrange("(kt p) m -> p kt m", p=128), [128, KT, DO], "wk")
            wv_r = load_bf16(px, wv.rearrange("(kt p) m -> p kt m", p=128), [128, KT, DO], "wv")
            xr = []
            for kt in range(KT):
                xr.append(load_bf16(px, xt[kt * 128:(kt + 1) * 128, :], [128, T], f"xr{kt}"))

            # ---- persistent activation tiles ----
            qt2 = [persist.tile([128, T], MMDT, tag=f"qt{mi}", name=f"qt{mi}") for mi in range(2)]
            kth = [persist.tile([128, T], MMDT, tag=f"kh{h}", name=f"kh{h}") for h in range(NH)]
            vp = persist.tile([128, TT, NH * 65], MMDT, tag="vp")
            at4 = [persist.tile([128, T], MMDT, tag=f"at{h}", name=f"at{h}") for h in range(NH)]
            for h in range(NH):
                lo, hi = ((64, 128) if h % 2 == 0 else (0, 64))
                nc.vector.memset(kth[h][lo:hi, :], 0.0)
                nc.vector.memset(at4[h][64:128, :], 0.0)

            # ---- emission helpers (generators advanced by the interleaver) ----
            def qk_group(w_r, mi, nb, dst, pool_tag="attn"):
                """One [128,512] projection psum group: 8 matmuls + biased copy."""
                pool = ps_s if pool_tag == "s" else ps_mm
                ps = pool.tile([128, 1024], F32, tag=pool_tag, name=f"ps_p{mi}_{nb}")
                for kt in range(KT):
                    nc.tensor.matmul(
                        ps[:, :512],
                        w_r[:, kt, mi * 128:(mi + 1) * 128],
                        xr[kt][:, nb * 512:(nb + 1) * 512],
                        start=(kt == 0),
                        stop=(kt == KT - 1),
                    )
                nc.vector.tensor_scalar_add(
                    dst[mi][:, nb * 512:(nb + 1) * 512], ps[:, :512], bq_sb[:, mi:mi + 1]
                )

            kt2 = [None, None]  # paired K^T staging (rows then split into kth)

            def build_kth(mi):
                # even head of the pair: rows 0:64 stay, odd head: rows 64:128
                h0, h1 = 2 * mi, 2 * mi + 1
                nc.vector.tensor_copy(out=kth[h0][0:64, :], in_=kt2[mi][0:64, :])
                nc.vector.tensor_copy(out=kth[h1][64:128, :], in_=kt2[mi][64:128, :])

            def v_group(tt):
                ps = ps_mm.tile([128, 1024], F32, tag="attn", name=f"ps_v{tt}")
                for kt in range(KT):
                    nc.tensor.matmul(
                        ps[:, :DO],
                        xr[kt][:, tt * 128:(tt + 1) * 128],
                        wv_r[:, kt, :],
                        start=(kt == 0),
                        stop=(kt == KT - 1),
                    )
                for h in range(NH):
                    nc.vector.tensor_tensor(
                        vp[:, tt, h * 65:h * 65 + 64],
                        ps[:, h * 64:(h + 1) * 64],
                        bv_sb[:, h * 64:(h + 1) * 64],
                        ADD,
                    )
                ones_ap = vp[:, tt, :].rearrange("p (h c) -> p h c", c=65)[:, :, 64:65]
                nc.gpsimd.tensor_copy(out=ones_ap, in_=ones_f[:, :, None])

            wo_r = []

            def load_wo():
                for hh in range(2):
                    r = ptp.tile([128, 2, OUT], MMDT, tag=f"wo{hh}", name=f"r_wo{hh}")
                    nc.sync.dma_start(out=r[0:64, :, :], in_=wo[:, 2 * hh:2 * hh + 2, :])
                    nc.vector.memset(r[64:128, :, :], 0.0)
                    wo_r.append(r)

            def emit_pv(h, attn_ps, kt, pt):
                for half in range(2):
                    nc.tensor.matmul(
                        attn_ps[:65, half * 512:(half + 1) * 512],
                        vp[:, kt, h * 65:(h + 1) * 65],
                        pt[:, half * 512:(half + 1) * 512],
                        start=(kt == 0),
                        stop=(kt == TT - 1),
                    )

            def normalize(qbp, h, attn_ps):
                # copy the psum rows out right away so the attn slot frees
                # before the reciprocal chain (DVE->DMA->DVE->Pool) drains
                den = cw.tile([65, 1024], F32, tag="den", name=f"den{qbp}_{h}")
                nc.vector.tensor_copy(out=den[64:65, :], in_=attn_ps[64:65, :])
                acopy = small.tile([64, 1024], F32, tag="acopy", name=f"ac{qbp}_{h}")
                nc.vector.tensor_copy(out=acopy[:], in_=attn_ps[0:64, :])
                d0 = cw.tile([1, 1024], F32, tag="d0", name=f"d0{qbp}_{h}")
                nc.sync.dma_start(out=d0[:], in_=den[64:65, :])
                rec = cw.tile([1, 1024], F32, tag="rec", name=f"rec{qbp}_{h}")
                nc.vector.reciprocal_approx_fast(rec[:], d0[:])
                rb = small.tile([64, 1024], F32, tag="rb", name=f"rb{qbp}_{h}")
                nc.gpsimd.partition_broadcast(rb[:], rec[:])
                nc.vector.tensor_tensor(
                    at4[h][0:64, qbp * 1024:(qbp + 1) * 1024],
                    acopy[:],
                    rb[:],
                    MULT,
                )

            def c_group(nt, tb):
                ps = ps_mm.tile([128, 1024], F32, tag="attn", name=f"ps_c{nt}_{tb}")
                for h in range(NH):
                    nc.tensor.matmul(
                        ps[:, :512],
                        wo_r[h // 2][:, h % 2, nt * 128:(nt + 1) * 128],
                        at4[h][:, tb * 512:(tb + 1) * 512],
                        start=(h == 0),
                        stop=(h == NH - 1),
                    )
                o_sb = stage.tile([128, 512], F32, tag="stage", name="o_sb")
                nc.vector.tensor_copy(out=o_sb[:], in_=ps[:, :512])
                nc.sync.dma_start(
                    out=outT[nt * 128:(nt + 1) * 128, tb * 512:(tb + 1) * 512],
                    in_=o_sb[:],
                )

            # ---- emission schedule ----
            # minimal upfront work for head (0,0), then ONE globally
            # software-pipelined stream over all 128 attention units: the
            # PV of unit u-1 is emitted inside unit u, across head
            # boundaries too (attn psum bufs=2 keeps both heads' tiles
            # alive at a transition), so the per-head pipeline never
            # drains. Filler work (remaining projections, first half of
            # the output projection) rides on non-transition units.
            kt2[0] = persist.tile([128, T], MMDT, tag="kt2a", name="kt2a")
            kt2[1] = persist.tile([128, T], MMDT, tag="kt2b", name="kt2b")
            for nb in range(2):
                qk_group(wq_r, 0, nb, qt2, pool_tag="s")
            for nb in range(NB):
                qk_group(wk_r, 0, nb, kt2, pool_tag="s")
            build_kth(0)
            v_group(0)

            era1 = [(lambda tt=i + 1: v_group(tt)) for i in range(15)] + [None]
            era2_items = [
                lambda: qk_group(wq_r, 0, 2, qt2),
                lambda: qk_group(wq_r, 0, 3, qt2),
                lambda: qk_group(wq_r, 1, 0, qt2),
                lambda: qk_group(wq_r, 1, 1, qt2),
                lambda: qk_group(wk_r, 1, 0, kt2),
                lambda: qk_group(wk_r, 1, 1, kt2),
                lambda: qk_group(wk_r, 1, 2, kt2),
                lambda: qk_group(wk_r, 1, 3, kt2),
                lambda: build_kth(1),
                load_wo,
                lambda: qk_group(wq_r, 1, 2, qt2),
                lambda: qk_group(wq_r, 1, 3, qt2),
            ]
            era2 = [None] + era2_items + [None]
            while len(era2) < 16:
                era2.append(None)
            c_work = [(nt, tb) for tb in range(2) for nt in range(OUT // 128)]

            units = [(qbp, h, kt) for qbp in range(2) for h in range(NH) for kt in range(TT)]
            attn_tiles = {}
            prev = None
            for idx, (qbp, h, kt) in enumerate(units):
                if kt == 0:
                    attn_tiles[(qbp, h)] = ps_mm.tile(
                        [128, 1024], F32, tag="attn", name=f"attn_{qbp}_{h}"
                    )
                s_ps = ps_s.tile([128, 1024], F32, tag="s", name=f"s_{qbp}_{h}_{kt}")
                for half in range(2):
                    nc.tensor.matmul(
                        s_ps[:, half * 512:(half + 1) * 512],
                        kth[h][:, kt * 128:(kt + 1) * 128],
                        qt2[h // 2][:, qbp * 1024 + half * 512:qbp * 1024 + (half + 1) * 512],
                        start=True,
                        stop=True,
                    )
                pt = ptp.tile([128, 1024], MMDT, tag="pt")
                nc.scalar.activation(
                    pt[:], s_ps[:], EXP, bias=bcol_sb[:, kt:kt + 1], scale=0.125
                )
                if prev is not None:
                    pq, ph, pk, ppt = prev
                    emit_pv(ph, attn_tiles[(pq, ph)], pk, ppt)
                    if pk == TT - 1:
                        normalize(pq, ph, attn_tiles.pop((pq, ph)))
                # filler work, away from head-transition units
                if idx < 16:
                    item = era1[idx]
                    if item is not None:
                        item()
                elif idx < 32:
                    item = era2[idx - 16]
                    if item is not None:
                        item()
                elif idx >= 64 and 1 <= kt <= 14 and c_work:
                    nt, tb = c_work.pop(0)
                    c_group(nt, tb)
                prev = (qbp, h, kt, pt)
            pq, ph, pk, ppt = prev
            emit_pv(ph, attn_tiles[(pq, ph)], pk, ppt)
            normalize(pq, ph, attn_tiles.pop((pq, ph)))

            while c_work:
                nt, tb = c_work.pop(0)
                c_group(nt, tb)
            for tb in range(2, NB):
                for nt in range(OUT // 128):
                    c_group(nt, tb)

            if dbg:
                def dump32(dst, src_ap, shape, nm):
                    t = stage.tile(shape, F32, tag="dump", name=f"dump_{nm}")
                    nc.vector.tensor_copy(out=t[:], in_=src_ap)
                    nc.sync.dma_start(out=dst, in_=t[:])
                for h in range(NH):
                    dump32(dbg["d_qt"][h], qt2[h // 2][(h % 2) * 64:(h % 2) * 64 + 64, :], [64, T], f"qt{h}")
                    kt_rows = kth[h][0:64, :] if h % 2 == 0 else kth[h][64:128, :]
                    dump32(dbg["d_kt"][h], kt_rows, [64, T], f"kt{h}")
                    dump32(dbg["d_at"][h], at4[h][0:64, :], [64, T], f"at{h}")
                for tt in range(TT):
                    dump32(dbg["d_vp"][:, tt, :], vp[:, tt, :], [128, NH * 65], f"vp{tt}")


def _build():
    nc = bacc.Bacc(trn_type="TRN2")
    build_attention(nc)
    nc.compile()
    return nc


def _get_nc():
    if "nc" not in _CACHE:
        _CACHE["nc"] = _build()
    return _CACHE["nc"]


def make_in_maps(x, W_q, b_q, W_k, W_v, b_v, W_o, bias):
    import ml_dtypes
    bf16 = ml_dtypes.bfloat16
    in_maps = []
    xtb = [np.ascontiguousarray(x[b].T.astype(bf16)) for b in range(B)]
    wqb = W_q.astype(bf16)
    wkb = W_k.astype(bf16)
    wvb = W_v.astype(bf16)
    wob = W_o.astype(bf16)
    for c in range(8):
        b, hg = divmod(c, 4)
        sl = slice(hg * DO, (hg + 1) * DO)
        in_maps.append({
            "xt": xtb[b],
            "wq": np.ascontiguousarray(wqb[:, sl]),
            "wk": np.ascontiguousarray(wkb[:, sl]),
            "wv": np.ascontiguousarray(wvb[:, sl]),
            "wo": np.ascontiguousarray(wob[sl, :].reshape(NH, DEPTH, OUT).transpose(1, 0, 2)),
            "bq2": np.ascontiguousarray(b_q[sl].reshape(2, 128).T),
            "bv": np.ascontiguousarray(b_v[sl]),
            "bcol": np.ascontiguousarray(bias.reshape(TT, 128).T),
        })
    return in_maps


def kernel(x, W_q, b_q, W_k, b_k, W_v, b_v, W_o, b_o, bias, **_ignored):
    x = np.asarray(x, dtype=np.float32)
    W_q = np.asarray(W_q, dtype=np.float32)
    W_k = np.asarray(W_k, dtype=np.float32)
    W_v = np.asarray(W_v, dtype=np.float32)
    W_o = np.asarray(W_o, dtype=np.float32)
    b_q = np.asarray(b_q, dtype=np.float32)
    b_v = np.asarray(b_v, dtype=np.float32)
    b_o = np.asarray(b_o, dtype=np.float32)
    bias = np.asarray(bias, dtype=np.float32)

    nc = _get_nc()
    in_maps = make_in_maps(x, W_q, b_q, W_k, W_v, b_v, W_o, bias)
    _CACHE["in_maps"] = in_maps
    res = run_bass_kernel_spmd(nc, in_maps, list(range(8)))
    out = np.zeros((B, T, OUT), dtype=np.float32)
    for c in range(8):
        out[c // 4] += res.results[c]["outT"].T
    out += b_o
    return out


# revision 50
# speedup vs baseline: 1.2564x; 1.0044x over previous
"""Multi-head attention (B=2, T=2048, D=OUT=1024, H=16) on 8 TRN2 NeuronCores.

Sharding: data-parallel over batch (2 groups) x tensor-parallel over heads
(4 groups of 4 heads). Core c handles batch c//4, heads (c%4)*4..(c%4)*4+4.
Each core computes Q^T/K^T/V for its head group, streams softmax(QK^T)V
in transposed layout (keys on partitions), and a partial output projection
through its W_o row block. The host sums the 4 partials per batch and adds
b_o.

Device-side layout notes:
- x is fed transposed ([D, T]) so Q^T/K^T come straight out of the PE.
- The softmax denominator rides along as a 65th ones-column in V', so one
  matmul produces both attn_unnorm^T and the per-query denominator.
- Softmax normalization multiplies attn^T (64 rows) by a partition-
  broadcast of 1/denom; every vector op keeps matching partition bases
  (the DVE cannot shift data across partitions).
- Matmul operands are bf16 (fp32 PSUM accumulation): full PE clock and
  fast weight load; fp32r measured 2x slower (the HAM clock gate never
  re-warms, and 4-byte LDWEIGHTS cannot use FWL).
- Every matmul contracts over K=128: K=64 matmuls do not register as PE
  activity for the HAM clock gate and run at half clock forever. The
  K-side operand of each attention matmul is a per-head tile zero-padded
  to 128 rows (even heads: data in rows 0:64; odd heads: rows 64:128,
  matching where the paired Q^T tile holds that head's rows).
- Emission is interleaved so the scalar engine's exp stream (the true
  bottleneck, ~146us) starts as soon as heads 0-1 are projected and the
  remaining projection/output matmuls fill the PE while exp runs.
"""

import numpy as np

import concourse.bass as bass
import concourse.mybir as mybir
import concourse.tile as tile
from concourse import bacc
from concourse.bass_utils import run_bass_kernel_spmd

B, T, D, OUT, H = 2, 2048, 1024, 1024, 16
DO = 256            # output columns per core (4 heads x 64)
DEPTH = 64
NH = 4              # heads per core
KT = D // 128       # 8 contraction tiles for the projections
TT = T // 128       # 16 key tiles
NB = T // 512       # 4 query/time blocks
F32 = mybir.dt.float32
BF16 = mybir.dt.bfloat16
MMDT = BF16
EXP = mybir.ActivationFunctionType.Exp
MULT = mybir.AluOpType.mult
ADD = mybir.AluOpType.add

_CACHE = {}


def build_attention(nc, dbg=None):
    """Emit the attention program. dbg, when given, is a dict of extra DRAM
    debug outputs to dump intermediates into."""
    xt = nc.declare_dram_parameter("xt", [D, T], MMDT, isOutput=False)
    wq = nc.declare_dram_parameter("wq", [D, DO], MMDT, isOutput=False)
    wk = nc.declare_dram_parameter("wk", [D, DO], MMDT, isOutput=False)
    wv = nc.declare_dram_parameter("wv", [D, DO], MMDT, isOutput=False)
    wo = nc.declare_dram_parameter("wo", [2 * 128, OUT], MMDT, isOutput=False)
    bq2 = nc.declare_dram_parameter("bq2", [128, 2], F32, isOutput=False)
    bv = nc.declare_dram_parameter("bv", [DO], F32, isOutput=False)
    bcol = nc.declare_dram_parameter("bcol", [128, TT], F32, isOutput=False)
    outT = nc.declare_dram_parameter("outT", [OUT, T], F32, isOutput=True)

    with tile.TileContext(nc) as tc:
        with (
            tc.tile_pool(name="cw", bufs=1) as cw,
            tc.tile_pool(name="stage", bufs=6) as stage,
            tc.tile_pool(name="persist", bufs=1) as persist,
            tc.tile_pool(name="small", bufs=2) as small,
            tc.tile_pool(name="ptp", bufs=(4 if dbg else 6)) as ptp,
            tc.tile_pool(name="px", bufs=1) as px,
            tc.tile_pool(name="ps_s", bufs=2, space="PSUM") as ps_s,
            tc.tile_pool(name="ps_mm", bufs=2, space="PSUM") as ps_mm,
        ):
            # ---- inputs (bf16 straight from the host) ----
            def load_bf16(pool, dram_ap, shape, tag, eng=None):
                r = pool.tile(shape, MMDT, tag=tag, name=f"r_{tag}")
                (eng or nc.sync).dma_start(out=r[:], in_=dram_ap)
                return r

            wq_r = load_bf16(px, wq.rearrange("(kt p) m -> p kt m", p=128), [128, KT, DO], "wq")
            xr = []
            for kt in range(KT):
                xr.append(load_bf16(px, xt[kt * 128:(kt + 1) * 128, :], [128, T], f"xr{kt}"))
            wk_r = load_bf16(px, wk.rearrange("(kt p) m -> p kt m", p=128), [128, KT, DO], "wk")
            wv_r = load_bf16(px, wv.rearrange("(kt p) m -> p kt m", p=128), [128, KT, DO], "wv")

            # ---- constants (off the startup critical path) ----
            bq_sb = cw.tile([128, 2], F32, tag="bq")
            nc.sync.dma_start(out=bq_sb[:], in_=bq2[:, :])
            bcol_sb = cw.tile([128, TT], F32, tag="bcol")
            nc.sync.dma_start(out=bcol_sb[:], in_=bcol[:, :])
            bv_sb = cw.tile([128, DO], F32, tag="bv")
            bv_ap = bv.ap()
            bv_bcast = bass.AP(tensor=bv_ap.tensor, offset=bv_ap.offset, ap=[[0, 128], [1, DO]])
            nc.sync.dma_start(out=bv_sb[:], in_=bv_bcast)
            ones_f = cw.tile([128, NH], F32, tag="ones")
            nc.vector.memset(ones_f[:], 1.0)

            # ---- persistent activation tiles ----
            qt2 = [persist.tile([128, T], MMDT, tag=f"qt{mi}", name=f"qt{mi}") for mi in range(2)]
            kth = [persist.tile([128, T], MMDT, tag=f"kh{h}", name=f"kh{h}") for h in range(NH)]
            vp = persist.tile([128, TT, NH * 65], MMDT, tag="vp")
            # attn output tiles hold a head PAIR (even head rows 0:64, odd
            # head rows 64:128) so the output projection contracts K=128 of
            # real data -- half the matmuls of a per-head padded layout
            at2p = [persist.tile([128, T], MMDT, tag=f"atp{p}", name=f"atp{p}") for p in range(2)]
            for h in range(NH):
                lo, hi = ((64, 128) if h % 2 == 0 else (0, 64))
                nc.vector.memset(kth[h][lo:hi, :], 0.0)

            # ---- emission helpers (generators advanced by the interleaver) ----
            def qk_group(w_r, mi, nb, dst, pool_tag="attn"):
                """One [128,512] projection psum group: 8 matmuls + biased copy."""
                pool = ps_s if pool_tag == "s" else ps_mm
                ps = pool.tile([128, 1024], F32, tag=pool_tag, name=f"ps_p{mi}_{nb}")
                for kt in range(KT):
                    nc.tensor.matmul(
                        ps[:, :512],
                        w_r[:, kt, mi * 128:(mi + 1) * 128],
                        xr[kt][:, nb * 512:(nb + 1) * 512],
                        start=(kt == 0),
                        stop=(kt == KT - 1),
                    )
                nc.vector.tensor_scalar_add(
                    dst[mi][:, nb * 512:(nb + 1) * 512], ps[:, :512], bq_sb[:, mi:mi + 1]
                )

            kt2 = [None, None]  # paired K^T staging (rows then split into kth)

            def build_kth(mi):
                # even head of the pair: rows 0:64 stay, odd head: rows 64:128
                h0, h1 = 2 * mi, 2 * mi + 1
                nc.vector.tensor_copy(out=kth[h0][0:64, :], in_=kt2[mi][0:64, :])
                nc.vector.tensor_copy(out=kth[h1][64:128, :], in_=kt2[mi][64:128, :])

            def v_group(tt):
                ps = ps_mm.tile([128, 1024], F32, tag="attn", name=f"ps_v{tt}")
                for kt in range(KT):
                    nc.tensor.matmul(
                        ps[:, :DO],
                        xr[kt][:, tt * 128:(tt + 1) * 128],
                        wv_r[:, kt, :],
                        start=(kt == 0),
                        stop=(kt == KT - 1),
                    )
                for h in range(NH):
                    nc.vector.tensor_tensor(
                        vp[:, tt, h * 65:h * 65 + 64],
                        ps[:, h * 64:(h + 1) * 64],
                        bv_sb[:, h * 64:(h + 1) * 64],
                        ADD,
                    )
                ones_ap = vp[:, tt, :].rearrange("p (h c) -> p h c", c=65)[:, :, 64:65]
                nc.gpsimd.tensor_copy(out=ones_ap, in_=ones_f[:, :, None])

            wo_r = []

            def load_wo():
                r = ptp.tile([128, 2, OUT], MMDT, tag="wo", name="r_wo")
                nc.sync.dma_start(out=r[:], in_=wo.rearrange("(j two p) n -> (two p) j n", two=2, p=64))
                wo_r.append(r)

            def emit_pv(h, attn_ps, kt, pt):
                for half in range(2):
                    nc.tensor.matmul(
                        attn_ps[:65, half * 512:(half + 1) * 512],
                        vp[:, kt, h * 65:(h + 1) * 65],
                        pt[:, half * 512:(half + 1) * 512],
                        start=(kt == 0),
                        stop=(kt == TT - 1),
                    )

            def normalize(qbp, h, attn_ps):
                # copy the psum rows out right away so the attn slot frees
                # before the reciprocal chain (DVE->DMA->DVE->Pool) drains
                den = cw.tile([65, 1024], F32, tag="den", name=f"den{qbp}_{h}")
                nc.vector.tensor_copy(out=den[64:65, :], in_=attn_ps[64:65, :])
                acopy = small.tile([64, 1024], F32, tag="acopy", name=f"ac{qbp}_{h}")
                nc.vector.tensor_copy(out=acopy[:], in_=attn_ps[0:64, :])
                d0 = cw.tile([1, 1024], F32, tag="d0", name=f"d0{qbp}_{h}")
                nc.sync.dma_start(out=d0[:], in_=den[64:65, :])
                rec = cw.tile([1, 1024], F32, tag="rec", name=f"rec{qbp}_{h}")
                nc.vector.reciprocal_approx_fast(rec[:], d0[:])
                rb = small.tile([64, 1024], F32, tag="rb", name=f"rb{qbp}_{h}")
                nc.gpsimd.partition_broadcast(rb[:], rec[:])
                sl = slice(qbp * 1024, (qbp + 1) * 1024)
                if h % 2 == 0:
                    nc.vector.tensor_tensor(at2p[h // 2][0:64, sl], acopy[:], rb[:], MULT)
                else:
                    atmp = small.tile([64, 1024], MMDT, tag="atmp", name=f"atmp{qbp}_{h}")
                    nc.vector.tensor_tensor(atmp[:], acopy[:], rb[:], MULT)
                    nc.sync.dma_start(out=at2p[h // 2][64:128, sl], in_=atmp[:])

            def c_group(nt, tb):
                ps = ps_mm.tile([128, 1024], F32, tag="attn", name=f"ps_c{nt}_{tb}")
                for j in range(2):
                    nc.tensor.matmul(
                        ps[:, :512],
                        wo_r[0][:, j, nt * 128:(nt + 1) * 128],
                        at2p[j][:, tb * 512:(tb + 1) * 512],
                        start=(j == 0),
                        stop=(j == 1),
                    )
                o_sb = stage.tile([128, 512], F32, tag="stage", name="o_sb")
                nc.vector.tensor_copy(out=o_sb[:], in_=ps[:, :512])
                nc.sync.dma_start(
                    out=outT[nt * 128:(nt + 1) * 128, tb * 512:(tb + 1) * 512],
                    in_=o_sb[:],
                )

            # ---- emission schedule ----
            # minimal upfront work for head (0,0), then ONE globally
            # software-pipelined stream over all 128 attention units: the
            # PV of unit u-1 is emitted inside unit u, across head
            # boundaries too (attn psum bufs=2 keeps both heads' tiles
            # alive at a transition), so the per-head pipeline never
            # drains. Filler work (remaining projections, first half of
            # the output projection) rides on non-transition units.
            kt2[0] = persist.tile([128, T], MMDT, tag="kt2a", name="kt2a")
            kt2[1] = persist.tile([128, T], MMDT, tag="kt2b", name="kt2b")
            for nb in range(2):
                qk_group(wq_r, 0, nb, qt2, pool_tag="s")
            for nb in range(NB):
                qk_group(wk_r, 0, nb, kt2, pool_tag="s")
            build_kth(0)
            v_group(0)

            era1 = [(lambda tt=i + 1: v_group(tt)) for i in range(15)] + [None]
            era2_items = [
                lambda: qk_group(wq_r, 0, 2, qt2),
                lambda: qk_group(wq_r, 0, 3, qt2),
                lambda: qk_group(wq_r, 1, 0, qt2),
                lambda: qk_group(wq_r, 1, 1, qt2),
                lambda: qk_group(wk_r, 1, 0, kt2),
                lambda: qk_group(wk_r, 1, 1, kt2),
                lambda: qk_group(wk_r, 1, 2, kt2),
                lambda: qk_group(wk_r, 1, 3, kt2),
                lambda: build_kth(1),
                load_wo,
                lambda: qk_group(wq_r, 1, 2, qt2),
                lambda: qk_group(wq_r, 1, 3, qt2),
            ]
            late_items = era2_items[-3:]           # load_wo, Q1nb2, Q1nb3
            era2 = [None] + era2_items[:-3] + [None]
            while len(era2) < 16:
                era2.append(None)
            era3 = [None] * 32                      # heads (0,2)/(0,3)
            era3[2], era3[6], era3[10] = late_items
            c_work = [(nt, tb) for tb in range(2) for nt in range(OUT // 128)]

            units = [(qbp, h, kt) for qbp in range(2) for h in range(NH) for kt in range(TT)]
            attn_tiles = {}
            prev = None
            for idx, (qbp, h, kt) in enumerate(units):
                if kt == 0:
                    attn_tiles[(qbp, h)] = ps_mm.tile(
                        [128, 1024], F32, tag="attn", name=f"attn_{qbp}_{h}"
                    )
                s_ps = ps_s.tile([128, 1024], F32, tag="s", name=f"s_{qbp}_{h}_{kt}")
                for half in range(2):
                    nc.tensor.matmul(
                        s_ps[:, half * 512:(half + 1) * 512],
                        kth[h][:, kt * 128:(kt + 1) * 128],
                        qt2[h // 2][:, qbp * 1024 + half * 512:qbp * 1024 + (half + 1) * 512],
                        start=True,
                        stop=True,
                    )
                pt = ptp.tile([128, 1024], MMDT, tag="pt")
                nc.scalar.activation(
                    pt[:], s_ps[:], EXP, bias=bcol_sb[:, kt:kt + 1], scale=0.125
                )
                if prev is not None:
                    pq, ph, pk, ppt = prev
                    emit_pv(ph, attn_tiles[(pq, ph)], pk, ppt)
                    if pk == TT - 1:
                        normalize(pq, ph, attn_tiles.pop((pq, ph)))
                # filler work, away from head-transition units
                if idx < 16:
                    item = era1[idx]
                    if item is not None:
                        item()
                elif idx < 32:
                    item = era2[idx - 16]
                    if item is not None:
                        item()
                elif idx < 64:
                    item = era3[idx - 32]
                    if item is not None:
                        item()
                elif idx >= 64 and 1 <= kt <= 14 and c_work:
                    nt, tb = c_work.pop(0)
                    c_group(nt, tb)
                prev = (qbp, h, kt, pt)
            pq, ph, pk, ppt = prev
            emit_pv(ph, attn_tiles[(pq, ph)], pk, ppt)
            normalize(pq, ph, attn_tiles.pop((pq, ph)))

            while c_work:
                nt, tb = c_work.pop(0)
                c_group(nt, tb)
            for tb in range(2, NB):
                for nt in range(OUT // 128):
                    c_group(nt, tb)

            if dbg:
                def dump32(dst, src_ap, shape, nm):
                    t = stage.tile(shape, F32, tag="dump", name=f"dump_{nm}")
                    nc.vector.tensor_copy(out=t[:], in_=src_ap)
                    nc.sync.dma_start(out=dst, in_=t[:])
                for h in range(NH):
                    dump32(dbg["d_qt"][h], qt2[h // 2][(h % 2) * 64:(h % 2) * 64 + 64, :], [64, T], f"qt{h}")
                    kt_rows = kth[h][0:64, :] if h % 2 == 0 else kth[h][64:128, :]
                    dump32(dbg["d_kt"][h], kt_rows, [64, T], f"kt{h}")
                    dump32(dbg["d_at"][h], at2p[h // 2][(h % 2) * 64:(h % 2) * 64 + 64, :], [64, T], f"at{h}")
                for tt in range(TT):
                    dump32(dbg["d_vp"][:, tt, :], vp[:, tt, :], [128, NH * 65], f"vp{tt}")


def _build():
    nc = bacc.Bacc(trn_type="TRN2")
    build_attention(nc)
    nc.compile()
    return nc


def _get_nc():
    if "nc" not in _CACHE:
        _CACHE["nc"] = _build()
    return _CACHE["nc"]


def make_in_maps(x, W_q, b_q, W_k, W_v, b_v, W_o, bias):
    import ml_dtypes
    bf16 = ml_dtypes.bfloat16
    in_maps = []
    xtb = [np.ascontiguousarray(x[b].T.astype(bf16)) for b in range(B)]
    wqb = W_q.astype(bf16)
    wkb = W_k.astype(bf16)
    wvb = W_v.astype(bf16)
    wob = W_o.astype(bf16)
    for c in range(8):
        b, hg = divmod(c, 4)
        sl = slice(hg * DO, (hg + 1) * DO)
        in_maps.append({
            "xt": xtb[b],
            "wq": np.ascontiguousarray(wqb[:, sl]),
            "wk": np.ascontiguousarray(wkb[:, sl]),
            "wv": np.ascontiguousarray(wvb[:, sl]),
            "wo": np.ascontiguousarray(wob[sl, :]),
            "bq2": np.ascontiguousarray(b_q[sl].reshape(2, 128).T),
            "bv": np.ascontiguousarray(b_v[sl]),
            "bcol": np.ascontiguousarray(bias.reshape(TT, 128).T),
        })
    return in_maps


def kernel(x, W_q, b_q, W_k, b_k, W_v, b_v, W_o, b_o, bias, **_ignored):
    x = np.asarray(x, dtype=np.float32)
    W_q = np.asarray(W_q, dtype=np.float32)
    W_k = np.asarray(W_k, dtype=np.float32)
    W_v = np.asarray(W_v, dtype=np.float32)
    W_o = np.asarray(W_o, dtype=np.float32)
    b_q = np.asarray(b_q, dtype=np.float32)
    b_v = np.asarray(b_v, dtype=np.float32)
    b_o = np.asarray(b_o, dtype=np.float32)
    bias = np.asarray(bias, dtype=np.float32)

    nc = _get_nc()
    in_maps = make_in_maps(x, W_q, b_q, W_k, W_v, b_v, W_o, bias)
    _CACHE["in_maps"] = in_maps
    res = run_bass_kernel_spmd(nc, in_maps, list(range(8)))
    out = np.zeros((B, T, OUT), dtype=np.float32)
    for c in range(8):
        out[c // 4] += res.results[c]["outT"].T
    out += b_o
    return out
